# revision 7
# baseline (speedup 1.0000x reference)
"""FBSNN loss kernel for Trainium2 (8 NeuronCores, pure data parallel).

Reference computation: 50-step Euler scheme over B=262144 paths; each step
evaluates a Q-control MLP and a Y-value MLP (2->64->64->1, relu) plus the
dY/dy JVP, accumulating mean squared BSDE residuals and a terminal loss.

Kernel architecture (per core, Bc = 32768 paths = 64 chunks x 512), v2
"pair-packed" design. Empirical cost model for this backend (measured):
engine op cost scales with processed elements at ~120 G elem/s for ACT/DVE
(64-partition tiles pay 2x per element), matmuls ~0.16us per 512-wide
output, gpsimd ~8 G elem/s (never use), DMA ~free, plus a large fixed
per-execution overhead. Therefore the kernel minimizes total elementwise
work and instruction-chain stalls:

  - Per-path state y kept dense as [64, 512] (partition = chunk).
  - Chunks processed in PAIRS (a=2g, b=2g+1): all hidden tiles are full
    [128, 512] (rows 0:64 = chunk a, 64:128 = chunk b), so DVE mask/jvp ops
    run at full 128-partition rate, halving their cost vs single-chunk.
  - L1 extracts both chunks' y rows via a [64,128] two-one-hot stationary
    operand per net (Y and Q evaluated as separate pair-matmuls).
  - L2 uses blockdiag(w2, w2) stationary operands; jvp v likewise.
  - L3 accumulates u (psum rows 0:64) and dt*q (rows 64:128) of ONE psum
    bank across all pairs; du accumulates into a second bank. One ACT
    eviction for u+qd, one for du.
  - 3-stage software pipeline over pairs (A: L1+relu1+mask1, B: L2+relu2+v,
    C: mask2+dh2+L3) so PE/ACT/DVE overlap; ACT (4 relus/pair) is the
    steady-state bottleneck at ~2.2us/pair.
  - relu-jvp folded into precomputed weights; matmuls run as float32r.
  - Loss accumulated on-chip via ACT Square accum_out into [64, 1];
    per-core partials summed on host (no collectives).
"""

import os
import sys

import numpy as np

sys.path.insert(0, "/opt/trn_rl_repo")

import concourse.bacc as bacc  # noqa: E402
import concourse.tile as tile  # noqa: E402
from concourse import mybir  # noqa: E402
from concourse.bass_utils import run_bass_kernel_spmd  # noqa: E402

DT = 0.01
SIGMA = 0.5
N_STEPS = 50
N_CORES = 8
B_TOTAL = 262144
N_CHUNKS = 64
NFREE = 512
BC = N_CHUNKS * NFREE  # paths per core

F32 = mybir.dt.float32
F32R = mybir.dt.float32r
BF16 = mybir.dt.bfloat16
RELU = mybir.ActivationFunctionType.Relu
SQUARE = mybir.ActivationFunctionType.Square
COPY = mybir.ActivationFunctionType.Copy
ADD = mybir.AluOpType.add
SUB = mybir.AluOpType.subtract
MULT = mybir.AluOpType.mult
IS_GT = mybir.AluOpType.is_gt
MAX = mybir.AluOpType.max

_CACHE = {}
_LAST_RES = None
ABL = os.environ.get("FBSNN_ABL", "")


def _r(ap):
    """float32r view of an fp32 AP (full-rate PE matmul)."""
    return ap.bitcast(F32R)


def _build(n_steps, n_chunks, nfree, y0val):
    """Trace the Bass kernel. Returns the Bass object."""
    nc = bacc.Bacc(None, target_bir_lowering=False)

    HH = 64
    H = 128
    NP = n_chunks // 2  # chunk pairs

    # ---- DRAM I/O ----
    dws = nc.dram_tensor("dws", [n_steps, n_chunks, nfree], F32, kind="ExternalInput")
    l1y = nc.dram_tensor("l1y", [n_chunks, NP * H], F32R, kind="ExternalInput")
    l1q = nc.dram_tensor("l1q", [n_chunks, NP * H], F32R, kind="ExternalInput")
    w2y = nc.dram_tensor("w2y", [H, H], BF16, kind="ExternalInput")
    w2q = nc.dram_tensor("w2q", [H, H], BF16, kind="ExternalInput")
    w2pp = nc.dram_tensor("w2pp", [H, H], BF16, kind="ExternalInput")
    ucp = nc.dram_tensor("ucp", [H, NP * HH], BF16, kind="ExternalInput")
    qcp = nc.dram_tensor("qcp", [H, NP * HH], BF16, kind="ExternalInput")
    c1y = nc.dram_tensor("c1y", [H, 64], F32, kind="ExternalInput")
    c1q = nc.dram_tensor("c1q", [H, 64], F32, kind="ExternalInput")
    b2y = nc.dram_tensor("b2y", [H, 1], F32, kind="ExternalInput")
    b2q = nc.dram_tensor("b2q", [H, 1], F32, kind="ExternalInput")
    scal = nc.dram_tensor("scal", [128, 8], F32, kind="ExternalInput")
    loss_out = nc.dram_tensor("loss_part", [n_chunks, 1], F32, kind="ExternalOutput")

    with tile.TileContext(nc) as tc:
        import contextlib

        with contextlib.ExitStack() as ctx:
            const = ctx.enter_context(tc.tile_pool(name="const", bufs=1))
            state = ctx.enter_context(tc.tile_pool(name="state", bufs=1))
            wb = 4 if "buf4" in ABL else 3
            work = ctx.enter_context(tc.tile_pool(name="work", bufs=wb))
            dwp = ctx.enter_context(tc.tile_pool(name="dwp", bufs=3))
            scr = ctx.enter_context(tc.tile_pool(name="scr", bufs=2))
            ps_a1y = ctx.enter_context(tc.tile_pool(name="ps_a1y", bufs=1, space="PSUM"))
            ps_a1q = ctx.enter_context(tc.tile_pool(name="ps_a1q", bufs=1, space="PSUM"))
            ps_a2y = ctx.enter_context(tc.tile_pool(name="ps_a2y", bufs=1, space="PSUM"))
            ps_a2q = ctx.enter_context(tc.tile_pool(name="ps_a2q", bufs=1, space="PSUM"))
            ps_v = ctx.enter_context(tc.tile_pool(name="ps_v", bufs=1, space="PSUM"))
            ps_u = ctx.enter_context(tc.tile_pool(name="ps_u", bufs=1, space="PSUM"))
            ps_qd = ctx.enter_context(tc.tile_pool(name="ps_qd", bufs=1, space="PSUM"))
            ps_du = ctx.enter_context(tc.tile_pool(name="ps_du", bufs=1, space="PSUM"))

            # ---- load constants to SBUF ----
            l1y_sb = const.tile([n_chunks, NP * H], F32R)
            l1q_sb = const.tile([n_chunks, NP * H], F32R)
            w2y_sb = const.tile([H, H], BF16)
            w2q_sb = const.tile([H, H], BF16)
            w2pp_sb = const.tile([H, H], BF16)
            ucp_sb = const.tile([H, NP * HH], BF16)
            qcp_sb = const.tile([H, NP * HH], BF16)
            c1y_sb = const.tile([H, 64], F32)
            c1q_sb = const.tile([H, 64], F32)
            b2y_sb = const.tile([H, 1], F32)
            b2q_sb = const.tile([H, 1], F32)
            scal_sb = const.tile([128, 8], F32)
            for dst, src in ((l1y_sb, l1y), (l1q_sb, l1q), (w2y_sb, w2y),
                             (w2q_sb, w2q), (w2pp_sb, w2pp), (ucp_sb, ucp),
                             (qcp_sb, qcp), (c1y_sb, c1y), (c1q_sb, c1q),
                             (b2y_sb, b2y), (b2q_sb, b2q), (scal_sb, scal)):
                nc.sync.dma_start(dst[:], src[:])

            # ---- persistent state ----
            y_sb = state.tile([n_chunks, nfree], F32)  # y - n*qb (shifted state)
            u_st = [state.tile([n_chunks, nfree], F32, tag=f"u{p}", name=f"u{p}")
                    for p in (0, 1)]
            qd_st = [state.tile([n_chunks, nfree], F32, tag=f"qd{p}", name=f"qd{p}")
                     for p in (0, 1)]
            du_st = [state.tile([n_chunks, nfree], F32, tag=f"du{p}", name=f"du{p}")
                     for p in (0, 1)]
            acc = state.tile([n_chunks, 1], F32, tag="acc", name="acc")
            sacc = state.tile([n_chunks, 1], F32, tag="sacc", name="sacc")

            y_init = scr.tile([n_chunks, nfree], F32, tag="sq", name="y_init")
            nc.vector.memset(y_init[:], float(y0val))
            nc.scalar.activation(_r(y_sb[:]), y_init[:], COPY)
            nc.vector.memset(acc[:], 0.0)

            def emit_eval(n, u_dst, qd_dst, du_dst):
                """Fused pair-packed Y+Q MLP eval (+ Y jvp) on dense y_sb.

                n: step index for L1 bias columns.
                Writes u / dt*q / du as dense [n_chunks, nfree] sbuf tiles.
                """
                by = c1y_sb[:, n:n + 1]
                bq = c1q_sb[:, n:n + 1]
                u_ps = ps_u.tile([n_chunks, nfree], F32, tag="ups", name="u_ps")
                qd_ps = ps_qd.tile([n_chunks, nfree], F32, tag="qdps", name="qd_ps")
                du_ps = ps_du.tile([n_chunks, nfree], F32, tag="dups", name="du_ps")

                st = {}  # per-pair live tiles

                def stage_a(g):
                    a1y = ps_a1y.tile([H, nfree], F32, tag="a1y", name="a1y")
                    a1q = ps_a1q.tile([H, nfree], F32, tag="a1q", name="a1q")
                    h1y = work.tile([H, nfree], BF16, tag="h1y", name="h1y")
                    h1q = work.tile([H, nfree], BF16, tag="h1q", name="h1q")
                    m1 = work.tile([H, nfree], BF16, tag="m1", name="m1")
                    nc.tensor.matmul(a1y[:], l1y_sb[:, g * H:(g + 1) * H], _r(y_sb[:]))
                    nc.tensor.matmul(a1q[:], l1q_sb[:, g * H:(g + 1) * H], _r(y_sb[:]))
                    nc.scalar.activation(h1y[:], a1y[:], RELU, bias=by)
                    nc.scalar.activation(h1q[:], a1q[:], RELU, bias=bq)
                    nc.vector.tensor_scalar(m1[:], h1y[:], 0.0, None, IS_GT)
                    st[g] = dict(h1y=h1y, h1q=h1q, m1=m1)

                def stage_b_mm(g):
                    s = st[g]
                    a2y = ps_a2y.tile([H, nfree], F32, tag="a2y", name="a2y")
                    a2q = ps_a2q.tile([H, nfree], F32, tag="a2q", name="a2q")
                    h2y = work.tile([H, nfree], BF16, tag="h2y", name="h2y")
                    h2q = work.tile([H, nfree], BF16, tag="h2q", name="h2q")
                    nc.tensor.matmul(a2y[:], w2y_sb[:], s["h1y"][:])
                    if "bal2" in ABL and g % 2 == 1:
                        nc.vector.tensor_scalar(h2y[:], a2y[:], b2y_sb[:, 0:1],
                                                0.0, ADD, MAX)
                    else:
                        nc.scalar.activation(h2y[:], a2y[:], RELU,
                                             bias=b2y_sb[:, 0:1])
                    nc.tensor.matmul(a2q[:], w2q_sb[:], s["h1q"][:])
                    if g % 2 == 0:
                        # balance: relu2Q on DVE (2-op ts: +bias, max 0)
                        nc.vector.tensor_scalar(h2q[:], a2q[:], b2q_sb[:, 0:1],
                                                0.0, ADD, MAX)
                    else:
                        nc.scalar.activation(h2q[:], a2q[:], RELU,
                                             bias=b2q_sb[:, 0:1])
                    s.update(h2y=h2y, h2q=h2q)

                def stage_b_v(g):
                    s = st[g]
                    v = ps_v.tile([H, nfree], F32, tag="v", name="v")
                    nc.tensor.matmul(v[:], w2pp_sb[:], s["m1"][:])
                    s.update(v=v)

                def stage_b(g):
                    stage_b_mm(g)
                    stage_b_v(g)

                def stage_c_dve(g):
                    s = st[g]
                    m2 = work.tile([H, nfree], BF16, tag="m2", name="m2")
                    dh2 = work.tile([H, nfree], BF16, tag="dh2", name="dh2")
                    nc.vector.tensor_scalar(m2[:], s["h2y"][:], 0.0, None, IS_GT)
                    nc.vector.tensor_tensor(dh2[:], s["v"][:], m2[:], MULT)
                    s["dh2"] = dh2

                def stage_c_pe(g):
                    s = st[g]
                    kw = dict(start=(g == 0), stop=(g == NP - 1),
                              skip_group_check=True)
                    nc.tensor.matmul(u_ps[:], ucp_sb[:, g * HH:(g + 1) * HH],
                                     s["h2y"][:], **kw)
                    nc.tensor.matmul(qd_ps[:], qcp_sb[:, g * HH:(g + 1) * HH],
                                     s["h2q"][:], **kw)
                    nc.tensor.matmul(du_ps[:], ucp_sb[:, g * HH:(g + 1) * HH],
                                     s["dh2"][:], **kw)
                    del st[g]

                if "ord2" in ABL:
                    for i in range(NP + 2):
                        if i >= 2:
                            stage_c_dve(i - 2)
                        if 1 <= i <= NP:
                            stage_b_mm(i - 1)
                        if i < NP:
                            stage_a(i)
                        if i >= 2:
                            stage_c_pe(i - 2)
                        if 1 <= i <= NP:
                            stage_b_v(i - 1)
                else:
                    for i in range(NP + 2):
                        if i >= 2:
                            stage_c_dve(i - 2)
                        if i < NP:
                            stage_a(i)
                        if i >= 2:
                            stage_c_pe(i - 2)
                        if 1 <= i <= NP:
                            stage_b(i - 1)

                # evict psum -> sbuf
                nc.scalar.activation(u_dst[:], u_ps[:], COPY)
                nc.scalar.activation(qd_dst[:], qd_ps[:], COPY)
                nc.scalar.activation(du_dst[:], du_ps[:], COPY)

            # scal layout: [sq_scale, sq_bias, n_steps*qb, Yb3, 0...]
            sq_scale = scal_sb[:n_chunks, 0:1]

            def emit_step(n, par):
                """One recurrence step: y update, eval at (t_{n+1}, y_{n+1}),
                residual accumulate. par = parity of n (src buffers)."""
                src, dst = par, 1 - par
                dw_t = dwp.tile([n_chunks, nfree], F32, tag="dw")
                nc.sync.dma_start(dw_t[:], dws[n, :, :])
                # y += dt*q ; y += sigma*dW   (qb drift folded into c1y/c1q)
                nc.vector.tensor_tensor(_r(y_sb[:]), y_sb[:], qd_st[src][:], ADD)
                nc.vector.tensor_tensor(_r(y_sb[:]), y_sb[:], dw_t[:], ADD)
                # eval at new point
                emit_eval(n + 1, u_st[dst], qd_st[dst], du_st[dst])
                # resid = (u1 - u0) + (0.5/dt)*(qd+qb)^2 - du0*dWs
                sq = scr.tile([n_chunks, nfree], F32, tag="sq")
                r1 = scr.tile([n_chunks, nfree], F32, tag="r1")
                r3 = scr.tile([n_chunks, nfree], F32, tag="r3")
                rr = scr.tile([n_chunks, nfree], F32, tag="rr")
                # sq = (qd*s + qb*s)^2 via ACT Square(scale, bias)
                nc.scalar.activation(
                    sq[:], qd_st[src][:], SQUARE,
                    bias=scal_sb[:n_chunks, 1:2], scale=sq_scale,
                )
                nc.vector.tensor_tensor(r1[:], u_st[dst][:], u_st[src][:], SUB)
                nc.vector.tensor_tensor(r3[:], du_st[src][:], dw_t[:], MULT)
                nc.vector.tensor_tensor(r1[:], r1[:], sq[:], ADD)
                nc.vector.tensor_tensor(r1[:], r1[:], r3[:], SUB)
                # acc += sum_f(resid^2): ACT Square w/ accum_out + tiny add
                nc.scalar.activation(rr[:], r1[:], SQUARE, accum_out=sacc[:])
                nc.vector.tensor_tensor(acc[:], acc[:], sacc[:], ADD)

            # ---- E_0 at (t_0, y_0) ----
            emit_eval(0, u_st[0], qd_st[0], du_st[0])

            # ---- main loop ----
            for n in range(n_steps):
                emit_step(n, n % 2)

            fin = n_steps % 2  # parity of final buffers
            # ---- terminal: acc += (u_N - y_N^2)^2 ----
            # y_N = y_sb + n_steps*qb ; term = u_N - y_N^2
            t1 = scr.tile([n_chunks, nfree], F32, tag="sq")
            t2 = scr.tile([n_chunks, nfree], F32, tag="r1")
            rr = scr.tile([n_chunks, nfree], F32, tag="rr")
            # t1 = (y + n*qb)^2
            nc.scalar.activation(t1[:], y_sb[:], SQUARE, bias=scal_sb[:n_chunks, 2:3])
            nc.vector.tensor_tensor(t2[:], u_st[fin][:], t1[:], SUB)
            # u_true = u_psum + Yb3 -> add via tensor_scalar
            nc.vector.tensor_scalar(t2[:], t2[:], scal_sb[:n_chunks, 3:4], None, ADD)
            nc.scalar.activation(rr[:], t2[:], SQUARE, accum_out=sacc[:])
            nc.vector.tensor_tensor(acc[:], acc[:], sacc[:], ADD)
            nc.sync.dma_start(loss_out[:], acc[:])

    nc.compile()
    return nc


def _consts(Yw1, Yb1, Yw2, Yb2, Yw3, Yb3, Qw1, Qb1, Qw2, Qb2, Qw3, Qb3,
            n_steps, n_chunks):
    """Host-side constant packing. All fp32 numpy."""
    HH, H = 64, 128
    NP = n_chunks // 2
    f = np.float32
    ycol = Yw1[1, :].astype(f)
    qcol = Qw1[1, :].astype(f)
    qb = f(DT) * Qb3.astype(f)[0]  # dt * Qb3

    l1y = np.zeros((n_chunks, NP * H), f)
    l1q = np.zeros((n_chunks, NP * H), f)
    for g in range(NP):
        a, b = 2 * g, 2 * g + 1
        l1y[a, g * H: g * H + HH] = ycol
        l1y[b, g * H + HH: (g + 1) * H] = ycol
        l1q[a, g * H: g * H + HH] = qcol
        l1q[b, g * H + HH: (g + 1) * H] = qcol

    def blockdiag2(m):
        out = np.zeros((H, H), f)
        out[:HH, :HH] = m
        out[HH:, HH:] = m
        return out

    w2y = blockdiag2(Yw2.astype(f))
    w2q = blockdiag2(Qw2.astype(f))
    w2pp = blockdiag2((ycol[:, None] * Yw2).astype(f))

    yw3 = Yw3[:, 0].astype(f)
    qw3 = Qw3[:, 0].astype(f)
    ucp = np.zeros((H, NP * HH), f)
    qcp = np.zeros((H, NP * HH), f)
    for g in range(NP):
        a, b = 2 * g, 2 * g + 1
        ucp[:HH, g * HH + a] = yw3
        ucp[HH:, g * HH + b] = yw3
        qcp[:HH, g * HH + a] = f(DT) * qw3
        qcp[HH:, g * HH + b] = f(DT) * qw3

    # t_n replicating reference's fp32 accumulation t += DT
    ts = np.zeros(n_steps + 1, f)
    t = f(0.0)
    for n in range(1, n_steps + 1):
        t = f(t + f(DT))
        ts[n] = t
    c1y = np.zeros((H, 64), f)
    c1q = np.zeros((H, 64), f)
    for n in range(n_steps + 1):
        # qb drift fold: y_state = y_true - n*qb  =>  bias += n*qb*w1col
        by = ts[n] * Yw1[0, :].astype(f) + Yb1.astype(f) + f(n) * qb * ycol
        bq = ts[n] * Qw1[0, :].astype(f) + Qb1.astype(f) + f(n) * qb * qcol
        c1y[:HH, n] = by
        c1y[HH:, n] = by
        c1q[:HH, n] = bq
        c1q[HH:, n] = bq

    s = f(np.sqrt(0.5 / DT))
    scal = np.zeros((128, 8), f)
    scal[:, 0] = s
    scal[:, 1] = s * qb
    scal[:, 2] = f(n_steps) * qb
    scal[:, 3] = Yb3.astype(f)[0]
    b2yc = np.concatenate([Yb2, Yb2]).astype(f).reshape(H, 1)
    b2qc = np.concatenate([Qb2, Qb2]).astype(f).reshape(H, 1)
    import ml_dtypes
    bf = ml_dtypes.bfloat16
    return dict(l1y=l1y, l1q=l1q, w2y=w2y.astype(bf), w2q=w2q.astype(bf),
                w2pp=w2pp.astype(bf), ucp=ucp.astype(bf), qcp=qcp.astype(bf),
                c1y=c1y, c1q=c1q, b2y=b2yc, b2q=b2qc, scal=scal)


def _run(dW, y0_init, weights, n_steps, n_cores, n_chunks, nfree,
         trace=False, tmpdir=None):
    f = np.float32
    B = dW.shape[1]
    bc = n_chunks * nfree
    assert B == n_cores * bc
    y0val = float(np.asarray(y0_init).reshape(-1)[0])

    key = (n_steps, n_chunks, nfree, y0val, ABL)
    if key not in _CACHE:
        _CACHE[key] = _build(n_steps, n_chunks, nfree, y0val)
    nc = _CACHE[key]

    cd = _consts(*weights, n_steps, n_chunks)
    dws = (f(SIGMA) * dW.reshape(n_steps, B)).astype(f)  # [S, B]

    in_maps = []
    for k in range(n_cores):
        m = dict(cd)
        m["dws"] = np.ascontiguousarray(
            dws[:, k * bc: (k + 1) * bc].reshape(n_steps, n_chunks, nfree))
        in_maps.append(m)

    global _LAST_RES
    res = run_bass_kernel_spmd(nc, in_maps, core_ids=list(range(n_cores)),
                               trace=trace, tmpdir=tmpdir)
    _LAST_RES = res
    total = f(0.0)
    for k in range(n_cores):
        total += res.results[k]["loss_part"].astype(np.float64).sum().astype(f)
    loss = np.float32(total / f(B))
    return np.asarray(loss, dtype=np.float32), res


def kernel(dW, y0_init, Yw1, Yb1, Yw2, Yb2, Yw3, Yb3,
           Qw1, Qb1, Qw2, Qb2, Qw3, Qb3):
    dW = np.asarray(dW, dtype=np.float32)
    weights = tuple(np.asarray(x, dtype=np.float32) for x in
                    (Yw1, Yb1, Yw2, Yb2, Yw3, Yb3, Qw1, Qb1, Qw2, Qb2, Qw3, Qb3))
    n_steps = dW.shape[0]
    B = dW.shape[1]
    # full-size path: 8 cores x 64 chunks x 512
    if B == B_TOTAL and n_steps == N_STEPS:
        out, _ = _run(dW, y0_init, weights, n_steps, N_CORES, N_CHUNKS, NFREE,
                      trace=bool(int(os.environ.get("FBSNN_TRACE", "0"))))
        return out
    # small/debug path: single core, scale chunks to B (pairing needs even)
    nfree = 512 if B % 512 == 0 else B
    n_chunks = B // nfree
    assert n_chunks % 2 == 0, "pair-packed kernel needs an even chunk count"
    out, _ = _run(dW, y0_init, weights, n_steps, 1, n_chunks, nfree)
    return out
